# revision 1
# baseline (speedup 1.0000x reference)
"""Single-head causal attention on 8 Trainium2 NeuronCores.

Problem: x[8, 2048, 1024] f32, per-head projections (H=64), causal
softmax attention scaled by C**-0.5.

Strategy: data-parallel over batch (1 batch element per core). Per core
everything is kept in transposed layout so no fp32 on-chip transposes of
large tensors are needed:

  - host pre-casts x to bf16 and pre-transposes to xT [C, T]
  - Q^T/K^T [64, T] computed with the weight matrices as the stationary
    operand; Q and K are packed into one PSUM tile via PE column tiling
  - the kernel is pipelined over T-chunks of 512: projections for chunk
    tj are followed immediately by attention for query chunk jq=tj, so
    TensorE (projections) and ScalarE (exp) work overlap across chunks
  - S^T tiles [128 Tk, 512 Tq] = (K^T chunk).T @ Q^T, two at a time via
    PE row tiling (contraction dim is only H=64) into one two-bank PSUM
    tile, so a single ScalarE exp covers both
  - softmax runs without max-subtraction (logits are O(1) by
    construction); causal mask via precomputed mask tiles on VectorE
  - P@V: V tiles [128 Tk, 65] carry an appended ones column, so the
    softmax denominator falls out of the same PSUM accumulation as the
    numerator
  - normalization (divide by denominator row) + final transpose on host

Outputs are returned as float32 [8, 2048, 64].
"""

import numpy as np
import ml_dtypes

import concourse.bass as bass
import concourse.mybir as mybir
import concourse.tile as tile
from concourse import bacc
from concourse.bass_utils import run_bass_kernel_spmd

B, T, C, H = 8, 2048, 1024, 64
N_CORES = 8
TQ = 512          # Tq chunk (one fp32 PSUM bank)
N_JQ = T // TQ    # 4
N_TK = T // 128   # 16
N_KC = C // 128   # 8  contraction chunks for projections

DT16 = mybir.dt.float16  # fp16: same PE/DVE/DMA speed as bf16, 8 more mantissa bits
F32 = mybir.dt.float32
AF = mybir.ActivationFunctionType

MASK_MODE = "dve"   # "pe": fold into PE accumulation; "dve": multiply on VectorE
PIPELINE = True     # weave proj(tj) emission with attn(tj-1) for PE overlap
PREFETCH = True     # issue xT DMAs one wave early
SPREAD_TJ0 = False  # spreading tj0 issues across queues measured worse
VPAIR = False       # V-pair packing measured worse under pipelining
MERGE34 = False     # merging attn(3) into wave 3 measured worse
WARMUP = False      # HAM pre-warm dummies measured worse

_CACHED_NC = None


def build_program(reps=1):
    nc = bacc.Bacc("TRN2", target_bir_lowering=False, debug=False,
                   num_devices=N_CORES)

    xT_d = nc.dram_tensor("xT", [C, T], DT16, kind="ExternalInput").ap()
    wqk_d = nc.dram_tensor("wqk", [128, N_KC, 128], DT16,
                           kind="ExternalInput").ap()
    wv_d = nc.dram_tensor("wv", [128, N_KC, H], DT16,
                          kind="ExternalInput").ap()
    bqk_d = nc.dram_tensor("bqk", [128, 1], F32, kind="ExternalInput").ap()
    bv_d = nc.dram_tensor("bv", [128, 1], F32, kind="ExternalInput").ap()
    # additive causal masks (0 where allowed, -1e30 where masked), one
    # [128, 512] tile per diagonal offset m*128
    masks_d = nc.dram_tensor("masks", [128, 4, TQ], DT16,
                             kind="ExternalInput").ap()
    id128_d = nc.dram_tensor("id128", [128, 128], DT16,
                             kind="ExternalInput").ap()
    masks01_d = nc.dram_tensor("masks01", [128, 2, 2 * TQ], DT16,
                               kind="ExternalInput").ap()
    ident_d = nc.dram_tensor("ident", [128, H], DT16, kind="ExternalInput").ap()
    y_d = nc.dram_tensor("y", [H + 1, T], F32, kind="ExternalOutput").ap()

    with tile.TileContext(nc) as tc:
        with (
            tc.tile_pool(name="const", bufs=1) as const,
            tc.tile_pool(name="data", bufs=min(reps, 2)) as data,
            tc.tile_pool(name="et", bufs=6) as et_pool,
            tc.tile_pool(name="ysb", bufs=2) as y_pool,
            tc.tile_pool(name="ps_proj", bufs=2, space="PSUM") as ps_proj,
            tc.tile_pool(name="ps_s", bufs=2, space="PSUM") as ps_s,
            tc.tile_pool(name="ps_o", bufs=2, space="PSUM") as ps_o,
        ):
            # ---- constants: loaded once ----------------------------------
            wqk_sb = const.tile([128, N_KC, 128], DT16, tag="wqk")
            wv_sb = const.tile([128, N_KC, H], DT16, tag="wv")
            bqk_sb = const.tile([128, 1], F32, tag="bqk")
            bv_sb = const.tile([128, 1], F32, tag="bv")
            masks_sb = const.tile([128, 4, TQ], DT16, tag="masks")
            masks01_sb = const.tile([128, 2, 2 * TQ], DT16, tag="masks01")
            id128_sb = const.tile([128, 128], DT16, tag="id128")
            ident_sb = const.tile([128, H], DT16, tag="ident")
            nc.scalar.dma_start(wqk_sb[:], wqk_d)

            for _rep in range(reps):
                # ---- per-iteration tiles ---------------------------------
                xT_sb = data.tile([128, N_KC, T], DT16, tag="xT")
                # Q2: rows 0:64 = Q^T, rows 64:128 = copy (for row tiling)
                # K2: rows 64:128 = K^T, rows 0:64 = copy
                q2_sb = data.tile([128, T], DT16, tag="q2")
                k2_sb = data.tile([128, T], DT16, tag="k2")
                vT_sb = data.tile([128, T // 2], DT16, tag="vT")
                v_sb = data.tile([128, N_TK, H + 1], DT16, tag="v")
                nc.vector.memset(v_sb[:], 1.0)

                if WARMUP and _rep == 0:
                    # PE idles ~5us waiting for the first x chunk; spend it
                    # warming the HAM clock gate (K=4/8 -> 8/8 needs ~3.4us
                    # of sustained activity) so proj(0) runs at 2.4 GHz
                    wsrc = data.tile([128, H], DT16, tag="warm")
                    nc.vector.memset(wsrc[:], 0.5)
                    wps = ps_proj.tile([128, H], F32, tag="proj",
                                       name="warm_ps")
                    for _w in range(48):
                        nc.tensor.matmul(wps[0:64, :], wsrc[:], wsrc[:],
                                         start=True, stop=True,
                                         tile_position=(0, 0),
                                         skip_group_check=True)

                def dma_thunk(tj, _rep=_rep, xT_sb=xT_sb):
                    sl = slice(tj * TQ, (tj + 1) * TQ)

                    def dmas():
                        if tj == 0 and SPREAD_TJ0:
                            # spread first-chunk issues across all three DMA
                            # queues: issue cost is ~0.6us serial per queue
                            nc.sync.dma_start(xT_sb[:, 0, sl], xT_d[0:128, sl])
                            nc.scalar.dma_start(xT_sb[:, 1, sl],
                                                xT_d[128:256, sl])
                            nc.sync.dma_start(
                                xT_sb[:, 2:4, sl],
                                xT_d[256:512, sl]
                                .rearrange("(o p) t -> p o t", p=128))
                            nc.scalar.dma_start(
                                xT_sb[:, 4:6, sl],
                                xT_d[512:768, sl]
                                .rearrange("(o p) t -> p o t", p=128))
                            nc.gpsimd.dma_start(
                                xT_sb[:, 6:8, sl],
                                xT_d[768:1024, sl]
                                .rearrange("(o p) t -> p o t", p=128))
                            if _rep == 0:
                                nc.scalar.dma_start(wv_sb[:], wv_d)
                                nc.scalar.dma_start(bqk_sb[:], bqk_d)
                                nc.scalar.dma_start(bv_sb[:], bv_d)
                                nc.scalar.dma_start(ident_sb[:], ident_d)
                                nc.scalar.dma_start(id128_sb[:], id128_d)
                                nc.gpsimd.dma_start(masks_sb[:], masks_d)
                                nc.gpsimd.dma_start(masks01_sb[:], masks01_d)
                            return
                        if tj == 0:
                            nc.sync.dma_start(xT_sb[:, 0, sl], xT_d[0:128, sl])
                            nc.sync.dma_start(xT_sb[:, 1, sl],
                                              xT_d[128:256, sl])
                            start_c2 = 1
                        else:
                            start_c2 = 0
                        for c2 in range(start_c2, N_KC // 2):
                            nc.sync.dma_start(
                                xT_sb[:, 2 * c2:2 * c2 + 2, sl],
                                xT_d[c2 * 256:(c2 + 1) * 256, sl]
                                .rearrange("(o p) t -> p o t", p=128))
                        if tj == 0 and _rep == 0:
                            nc.scalar.dma_start(wv_sb[:], wv_d)
                            nc.scalar.dma_start(bqk_sb[:], bqk_d)
                            nc.scalar.dma_start(bv_sb[:], bv_d)
                            nc.scalar.dma_start(ident_sb[:], ident_d)
                            nc.scalar.dma_start(id128_sb[:], id128_d)
                            nc.gpsimd.dma_start(masks_sb[:], masks_d)
                            nc.gpsimd.dma_start(masks01_sb[:], masks01_d)
                    return dmas

                def proj_thunks(tj, xT_sb=xT_sb, q2_sb=q2_sb,
                                k2_sb=k2_sb, vT_sb=vT_sb, v_sb=v_sb):
                    sl = slice(tj * TQ, (tj + 1) * TQ)
                    st = {}
                    th = []

                    def qk_mm(c):
                        if c == 0:
                            st["ps"] = ps_proj.tile([128, TQ], F32, tag="proj", name="ps_qk")
                        ps = st["ps"]
                        nc.tensor.matmul(ps[0:64, :], wqk_sb[:, c, 0:64],
                                         xT_sb[:, c, sl],
                                         start=(c == 0), stop=(c == N_KC - 1),
                                         tile_position=(0, 0),
                                         skip_group_check=True)
                        nc.tensor.matmul(ps[64:128, :], wqk_sb[:, c, 64:128],
                                         xT_sb[:, c, sl],
                                         start=(c == 0), stop=(c == N_KC - 1),
                                         tile_position=(0, 64),
                                         skip_group_check=True)
                    for c in range(N_KC):
                        th.append(lambda c=c: qk_mm(c))

                    def qk_epi():
                        ps = st["ps"]
                        nc.vector.tensor_scalar_add(q2_sb[0:64, sl], ps[0:64, :],
                                                    bqk_sb[0:64, :])
                        nc.vector.tensor_scalar_add(k2_sb[64:128, sl],
                                                    ps[64:128, :],
                                                    bqk_sb[64:128, :])
                        nc.gpsimd.tensor_copy(q2_sb[64:128, sl],
                                              q2_sb[0:64, sl])
                        nc.gpsimd.tensor_copy(k2_sb[0:64, sl],
                                              k2_sb[64:128, sl])
                    th.append(qk_epi)

                    def v_mm(c):
                        # column-tiled pair: chunk tj in rows 0:64, chunk
                        # tj+1 in rows 64:128
                        if c == 0:
                            st["psv"] = ps_proj.tile([128, TQ], F32,
                                                     tag="proj", name="ps_v")
                        sl_b = slice((tj + 1) * TQ, (tj + 2) * TQ)
                        nc.tensor.matmul(st["psv"][0:64, :], wv_sb[:, c, :],
                                         xT_sb[:, c, sl],
                                         start=(c == 0), stop=(c == N_KC - 1),
                                         tile_position=(0, 0),
                                         skip_group_check=True)
                        nc.tensor.matmul(st["psv"][64:128, :], wv_sb[:, c, :],
                                         xT_sb[:, c, sl_b],
                                         start=(c == 0), stop=(c == N_KC - 1),
                                         tile_position=(0, 64),
                                         skip_group_check=True)
                    def v_mm_single(c):
                        if c == 0:
                            st["psv"] = ps_proj.tile([128, TQ], F32,
                                                     tag="proj", name="ps_v1")
                        nc.tensor.matmul(st["psv"][64 * (tj % 2):
                                                   64 * (tj % 2) + 64, :],
                                         wv_sb[:, c, :], xT_sb[:, c, sl],
                                         start=(c == 0), stop=(c == N_KC - 1),
                                         tile_position=(0, 64 * (tj % 2)))

                    if VPAIR and tj % 2 == 0:
                        for c in range(N_KC):
                            th.append(lambda c=c: v_mm(c))

                        def v_epi():
                            nc.vector.tensor_scalar_add(
                                vT_sb[:, (tj // 2) * TQ:(tj // 2 + 1) * TQ],
                                st["psv"][:], bv_sb[:])
                        th.append(v_epi)
                    elif not VPAIR:
                        for c in range(N_KC):
                            th.append(lambda c=c: v_mm_single(c))

                        def v_epi1():
                            half = slice(64 * (tj % 2), 64 * (tj % 2) + 64)
                            nc.vector.tensor_scalar_add(
                                vT_sb[half, (tj // 2) * TQ:(tj // 2 + 1) * TQ],
                                st["psv"][half, :], bv_sb[half, :])
                        th.append(v_epi1)

                    def v_tr(r):
                        i = tj * 4 + r
                        src = vT_sb[64 * (tj % 2):64 * (tj % 2) + 64,
                                    (tj // 2) * TQ + r * 128:
                                    (tj // 2) * TQ + (r + 1) * 128]
                        pt = ps_proj.tile([128, H], DT16, tag="proj")
                        nc.tensor.transpose(
                            pt, src,
                            ident_sb[64 * (tj % 2):64 * (tj % 2) + 64, :])
                        nc.vector.tensor_copy(v_sb[:, i, 0:H], pt)
                    for r in range(4):
                        th.append(lambda r=r: v_tr(r))
                    return th

                def attn_thunks(jq, q2_sb=q2_sb, k2_sb=k2_sb, v_sb=v_sb):
                    sl = slice(jq * TQ, (jq + 1) * TQ)
                    n_ik = (jq + 1) * 4
                    st = {}
                    th = []
                    q_lo = q2_sb[0:64, sl]
                    q_hi = q2_sb[64:128, sl]

                    def s_pair(s):
                        if s == 0:
                            st["po"] = ps_o.tile([H + 1, TQ], F32, tag="out", name="po")
                        ps2 = ps_s.tile([128, 2 * TQ], F32, tag="s")
                        st["ps2"] = ps2
                        for half, ik in ((0, 2 * s), (1, 2 * s + 1)):
                            m = ik - jq * 4
                            pe_mask = MASK_MODE == "pe" and m >= 0
                            sl2 = slice(half * TQ, (half + 1) * TQ)
                            klo = slice(64 * half, 64 * half + 64)
                            nc.tensor.matmul(ps2[:, sl2],
                                             k2_sb[klo, ik * 128:(ik + 1) * 128],
                                             q_lo if half == 0 else q_hi,
                                             start=True, stop=not pe_mask,
                                             tile_position=(64 * half, 0))
                            if pe_mask:
                                nc.tensor.matmul(ps2[:, sl2], id128_sb[:],
                                                 masks_sb[:, m, :],
                                                 start=False, stop=True,
                                                 tile_position=(0, 0))

                    def exp_pair(s):
                        eT = et_pool.tile([128, 2 * TQ], DT16, tag="et",
                                          name="eT")
                        st["eT"] = eT
                        nc.scalar.activation(eT[:], st["ps2"][:], AF.Exp)
                        if MASK_MODE == "dve" and 2 * s >= jq * 4:
                            nc.vector.tensor_mul(
                                eT[:], eT[:],
                                masks01_sb[:, (2 * s - jq * 4) // 2, :])

                    def pv_pair(s):
                        ik_a, ik_b = 2 * s, 2 * s + 1
                        eT = st["eT"]
                        nc.tensor.matmul(st["po"], v_sb[:, ik_a, :], eT[:, 0:TQ],
                                         start=(ik_a == 0), stop=False)
                        nc.tensor.matmul(st["po"], v_sb[:, ik_b, :],
                                         eT[:, TQ:2 * TQ],
                                         start=False, stop=(ik_b == n_ik - 1))

                    for s in range(n_ik // 2):
                        th.append(lambda s=s: (s_pair(s), exp_pair(s)))
                        th.append(lambda s=s: pv_pair(s))

                    def yout():
                        y_sb = y_pool.tile([H + 1, TQ], F32, tag="ysb")
                        nc.vector.tensor_copy(y_sb[:], st["po"])
                        nc.sync.dma_start(y_d[:, sl], y_sb[:])
                    th.append(yout)
                    return th

                def weave(a, b, f=1.0):
                    # f > 1 drains `a` proportionally faster (needed when b
                    # contains ops that depend on late items of a)
                    out, i, j = [], 0, 0
                    na, nb = len(a), len(b)
                    while i < na or j < nb:
                        if j >= nb or (i < na and i * nb <= j * na * f):
                            out.append(a[i]); i += 1
                        else:
                            out.append(b[j]); j += 1
                    return out

                if PIPELINE:
                    # software pipeline: weave proj(w) with attn(w-1) so the
                    # in-order TensorE stream always has independent work;
                    # xT loads for wave w+1 are issued early in wave w so
                    # wave boundaries never head-of-line block on DMA
                    n_waves = N_JQ if MERGE34 else N_JQ + 1
                    for wave in range(n_waves):
                        pa = proj_thunks(wave) if wave < N_JQ else []
                        if PREFETCH:
                            if wave == 0:
                                pa.insert(0, dma_thunk(0))
                            if wave + 1 < N_JQ:
                                pa.insert(min(2, len(pa)), dma_thunk(wave + 1))
                        elif wave < N_JQ:
                            pa.insert(0, dma_thunk(wave))
                        aa = attn_thunks(wave - 1) if wave >= 1 else []
                        fac = 1.0
                        if MERGE34 and wave == N_JQ - 1:
                            aa = aa + attn_thunks(N_JQ - 1)
                            # drain proj faster: attn(3)'s last PVs need the
                            # final v transposes emitted first
                            fac = 1.35
                        for t in weave(pa, aa, fac):
                            t()
                else:
                    for tj in range(N_JQ):
                        th = [dma_thunk(tj)] + proj_thunks(tj)
                        for t in th:
                            t()
                        for t in attn_thunks(tj):
                            t()

    nc.compile()
    return nc


def prepare_in_maps(x, wq, bq, wk, bk, wv, bv):
    f16 = np.float16
    f32 = np.float32
    sc = f32(C) ** -0.5
    xT = np.ascontiguousarray(
        np.asarray(x).astype(f16).transpose(0, 2, 1))          # [B, C, T]
    wqk = np.concatenate([np.asarray(wq) * sc, np.asarray(wk)], 1).astype(f16)
    # pre-stripe [C, M] -> [p, C//128, M] so the SBUF load is contiguous
    wqk = np.ascontiguousarray(wqk.reshape(N_KC, 128, 128).transpose(1, 0, 2))
    wv_c = np.asarray(wv).astype(f16)
    wv_c = np.ascontiguousarray(wv_c.reshape(N_KC, 128, H).transpose(1, 0, 2))
    bqk = np.concatenate([np.asarray(bq) * sc, np.asarray(bk)]) \
        .astype(f32).reshape(128, 1)
    bv_c = np.concatenate([np.asarray(bv), np.asarray(bv)]).astype(f32).reshape(128, 1)
    tk_i = np.arange(128)[:, None]
    tq_i = np.arange(TQ)[None, :]
    masks = np.ascontiguousarray(np.stack(
        [np.where(tq_i >= tk_i + m * 128, 0.0, -60000.0) for m in range(4)],
        1).astype(f16))
    single = [(tq_i >= tk_i + m * 128) for m in range(4)]
    masks01 = np.ascontiguousarray(np.stack(
        [np.concatenate([single[0], single[1]], 1),
         np.concatenate([single[2], single[3]], 1)], 1).astype(f16))
    ident = np.ascontiguousarray(np.concatenate([np.eye(H), np.eye(H)], 0).astype(f16))
    id128 = np.eye(128, dtype=f16)
    shared = {"wqk": wqk, "wv": wv_c, "bqk": bqk, "bv": bv_c,
              "masks": masks, "masks01": masks01, "ident": ident,
              "id128": id128}
    return [{"xT": np.ascontiguousarray(xT[b]), **shared} for b in range(B)]


def postprocess(ys):
    out = np.empty((B, T, H), np.float32)
    for b, y in enumerate(ys):
        out[b] = (y[:H] / y[H:H + 1]).T
    return out


def kernel(**inputs):
    global _CACHED_NC
    if _CACHED_NC is None:
        _CACHED_NC = build_program(reps=1)
    nc = _CACHED_NC
    in_maps = prepare_in_maps(
        inputs["x"], inputs["wq"], inputs["bq"], inputs["wk"],
        inputs["bk"], inputs["wv"], inputs["bv"])
    res = run_bass_kernel_spmd(nc, in_maps, core_ids=list(range(N_CORES)))
    return postprocess([r["y"] for r in res.results])


if __name__ == "__main__":
    rng = np.random.default_rng(0)
    demo = {
        "x": rng.standard_normal((B, T, C), dtype=np.float32),
        "wq": rng.standard_normal((C, H), dtype=np.float32) * 0.02,
        "bq": rng.standard_normal((H,), dtype=np.float32) * 0.02,
        "wk": rng.standard_normal((C, H), dtype=np.float32) * 0.02,
        "bk": rng.standard_normal((H,), dtype=np.float32) * 0.02,
        "wv": rng.standard_normal((C, H), dtype=np.float32) * 0.02,
        "bv": rng.standard_normal((H,), dtype=np.float32) * 0.02,
    }
    out = kernel(**demo)
    print("kernel output:", out.shape, out.dtype)



# revision 3
# speedup vs baseline: 1.0116x; 1.0116x over previous
"""Single-head causal attention on 8 Trainium2 NeuronCores.

Problem: x[8, 2048, 1024] f32, per-head projections (H=64), causal
softmax attention scaled by C**-0.5.

Strategy: data-parallel over batch (1 batch element per core). Per core
everything is kept in transposed layout so no fp32 on-chip transposes of
large tensors are needed:

  - host pre-casts x to fp16 and prepacks to wave-major xw
    [128, 4 waves, 8 c-chunks, 512 tok] so each wave loads as one DMA
    with 8KB contiguous partition lines (vs 1KB lines = 2-5x slower)
  - ~24 dummy matmuls sized to end when the first x data lands keep the
    PE HAM clock-gate warm (K=8/8, 2.4 GHz) for the real projections
  - Q^T/K^T [64, T] computed with [wq|wk] as stationary into one PSUM
    tile via PE column tiling; epilogue writes qk=[Q;K] in one 128-lane
    DVE op (bias fused), plus two small copies to build kq=[K;Q] for
    the row-tiled S matmuls
  - V^T for two token chunks is computed in one column-tiled pass
    (different moving streams -> both column groups run concurrently)
  - kernel pipelined over T-chunks of 512: projections for chunk tj
    weave with attention for query chunk tj-1
  - S^T tiles [128 Tk, 512 Tq] = (K^T chunk).T @ Q^T, two at a time via
    PE row tiling into one two-bank PSUM tile, single ScalarE exp per
    pair; causal mask via 0/1 mask multiply on VectorE
  - P@V: V tiles [128 Tk, 65] carry an appended ones column, so the
    softmax denominator falls out of the same PSUM accumulation
  - y returned as f16 [65, T]; normalization + transpose on host

Outputs are returned as float32 [8, 2048, 64].
"""

import numpy as np

import concourse.bass as bass
import concourse.mybir as mybir
import concourse.tile as tile
from concourse import bacc
from concourse.bass_utils import run_bass_kernel_spmd

B, T, C, H = 8, 2048, 1024, 64
N_CORES = 8
TQ = 512          # Tq chunk (one fp32 PSUM bank)
N_JQ = T // TQ    # 4
N_TK = T // 128   # 16
N_KC = C // 128   # 8  contraction chunks for projections

DT16 = mybir.dt.float16
F32 = mybir.dt.float32
AF = mybir.ActivationFunctionType

W_WARM = 24       # HAM warmup dummy matmuls (512 cols each)

_CACHED_NC = None


def build_program(reps=1):
    nc = bacc.Bacc("TRN2", target_bir_lowering=False, debug=False,
                   num_devices=N_CORES)

    xw_d = nc.dram_tensor("xw", [128, N_JQ, N_KC, TQ], DT16,
                          kind="ExternalInput").ap()
    wqk_d = nc.dram_tensor("wqk", [128, N_KC, 128], DT16,
                           kind="ExternalInput").ap()
    wv_d = nc.dram_tensor("wv", [128, N_KC, H], DT16,
                          kind="ExternalInput").ap()
    bqk_d = nc.dram_tensor("bqk", [128, 1], F32, kind="ExternalInput").ap()
    bv_d = nc.dram_tensor("bv", [128, 1], F32, kind="ExternalInput").ap()
    # 0/1 causal masks for the two diagonal pairs, [128, 2, 2*TQ]
    masks01_d = nc.dram_tensor("masks01", [128, 2, 2 * TQ], DT16,
                               kind="ExternalInput").ap()
    ident_d = nc.dram_tensor("ident", [128, H], DT16, kind="ExternalInput").ap()
    y_d = nc.dram_tensor("y", [H + 1, T], DT16, kind="ExternalOutput").ap()

    with tile.TileContext(nc) as tc:
        with (
            tc.tile_pool(name="const", bufs=1) as const,
            tc.tile_pool(name="data", bufs=1) as data,
            tc.tile_pool(name="et", bufs=6) as et_pool,
            tc.tile_pool(name="ysb", bufs=2) as y_pool,
            tc.tile_pool(name="ps_proj", bufs=2, space="PSUM") as ps_proj,
            tc.tile_pool(name="ps_s", bufs=2, space="PSUM") as ps_s,
            tc.tile_pool(name="ps_o", bufs=2, space="PSUM") as ps_o,
        ):
            # ---- constants ----------------------------------------------
            wqk_sb = const.tile([128, N_KC, 128], DT16, tag="wqk")
            wv_sb = const.tile([128, N_KC, H], DT16, tag="wv")
            bqk_sb = const.tile([128, 1], F32, tag="bqk")
            bv_sb = const.tile([128, 1], F32, tag="bv")
            masks01_sb = const.tile([128, 2, 2 * TQ], DT16, tag="masks01")
            ident_sb = const.tile([128, H], DT16, tag="ident")
            warm_sb = const.tile([128, TQ], DT16, tag="warm")

            # const loads: gpsimd queue (otherwise idle); priority order
            nc.gpsimd.dma_start(wqk_sb[:], wqk_d)
            nc.gpsimd.dma_start(wv_sb[:], wv_d)
            nc.gpsimd.dma_start(ident_sb[:], ident_d)
            nc.gpsimd.dma_start(bqk_sb[:], bqk_d)
            nc.gpsimd.dma_start(bv_sb[:], bv_d)
            nc.gpsimd.dma_start(masks01_sb[:], masks01_d)

            # ---- per-iteration tiles ------------------------------------
            # wave-major x: [128, wave, c, tok]
            xT_sb = data.tile([128, N_JQ, N_KC, TQ], DT16, tag="xT")
            qk_sb = data.tile([128, T], DT16, tag="qk")   # rows 0:64 Q, 64:128 K
            kq_sb = data.tile([128, T], DT16, tag="kq")   # rows 0:64 K, 64:128 Q
            vT_sb = data.tile([128, 2, TQ], DT16, tag="vT")
            v_sb = data.tile([128, N_TK, H + 1], DT16, tag="v")
            nc.vector.memset(v_sb[:], 1.0)
            nc.vector.memset(warm_sb[:], 0.5)

            # ---- x DMAs -------------------------------------------------
            def dma_x(wave):
                def go():
                    if wave == 0:
                        nc.sync.dma_start(xT_sb[:, 0, 0:4], xw_d[:, 0, 0:4])
                        nc.scalar.dma_start(xT_sb[:, 0, 4:8], xw_d[:, 0, 4:8])
                    elif wave == 2:
                        nc.scalar.dma_start(xT_sb[:, wave], xw_d[:, wave])
                    else:
                        nc.sync.dma_start(xT_sb[:, wave], xw_d[:, wave])
                return go

            # ---- HAM warmup: keep PE busy until first x data lands ------
            def warmup():
                wps = ps_proj.tile([128, TQ], F32, tag="proj", name="warm_ps")
                for _w in range(W_WARM):
                    nc.tensor.matmul(wps[:], warm_sb[:, 0:128], warm_sb[:],
                                     start=True, stop=True,
                                     skip_group_check=True)

            # ---- projections -------------------------------------------
            def qk_thunks(tj):
                sl = slice(tj * TQ, (tj + 1) * TQ)
                st = {}
                th = []

                def qk_mm(c):
                    if c == 0:
                        st["ps"] = ps_proj.tile([128, TQ], F32, tag="proj",
                                                name="ps_qk")
                    ps = st["ps"]
                    nc.tensor.matmul(ps[0:64, :], wqk_sb[:, c, 0:64],
                                     xT_sb[:, tj, c, :],
                                     start=(c == 0), stop=(c == N_KC - 1),
                                     tile_position=(0, 0),
                                     skip_group_check=True)
                    nc.tensor.matmul(ps[64:128, :], wqk_sb[:, c, 64:128],
                                     xT_sb[:, tj, c, :],
                                     start=(c == 0), stop=(c == N_KC - 1),
                                     tile_position=(0, 64),
                                     skip_group_check=True)
                for c in range(N_KC):
                    th.append(lambda c=c: qk_mm(c))

                def qk_epi():
                    nc.vector.tensor_scalar_add(qk_sb[:, sl], st["ps"][:],
                                                bqk_sb[:])
                    nc.vector.tensor_copy(kq_sb[0:64, sl], qk_sb[64:128, sl])
                    nc.vector.tensor_copy(kq_sb[64:128, sl], qk_sb[0:64, sl])
                th.append(qk_epi)
                return th

            def vpair_thunks(p):
                # V for token chunks 2p (rows 0:64) and 2p+1 (rows 64:128)
                st = {}
                th = []

                def v_mm(c):
                    if c == 0:
                        st["psv"] = ps_proj.tile([128, TQ], F32, tag="proj",
                                                 name="ps_v")
                    nc.tensor.matmul(st["psv"][0:64, :], wv_sb[:, c, :],
                                     xT_sb[:, 2 * p, c, :],
                                     start=(c == 0), stop=(c == N_KC - 1),
                                     tile_position=(0, 0),
                                     skip_group_check=True)
                    nc.tensor.matmul(st["psv"][64:128, :], wv_sb[:, c, :],
                                     xT_sb[:, 2 * p + 1, c, :],
                                     start=(c == 0), stop=(c == N_KC - 1),
                                     tile_position=(0, 64),
                                     skip_group_check=True)
                for c in range(N_KC):
                    th.append(lambda c=c: v_mm(c))

                def v_epi():
                    nc.vector.tensor_scalar_add(vT_sb[:, p, :], st["psv"][:],
                                                bv_sb[:])
                th.append(v_epi)

                def v_tr(r):
                    i = p * 8 + r
                    half = r // 4       # 0: chunk 2p, 1: chunk 2p+1
                    src = vT_sb[64 * half:64 * half + 64, p,
                                (r % 4) * 128:(r % 4 + 1) * 128]
                    pt = ps_proj.tile([128, H], DT16, tag="proj")
                    nc.tensor.transpose(
                        pt, src, ident_sb[64 * half:64 * half + 64, :])
                    nc.vector.tensor_copy(v_sb[:, i, 0:H], pt)
                for r in range(8):
                    th.append(lambda r=r: v_tr(r))
                return th

            # ---- attention ---------------------------------------------
            def attn_thunks(jq):
                sl = slice(jq * TQ, (jq + 1) * TQ)
                n_ik = (jq + 1) * 4
                st = {}
                th = []

                def s_pair(s):
                    if s == 0:
                        st["po"] = ps_o.tile([H + 1, TQ], F32, tag="out",
                                             name="po")
                    ps2 = ps_s.tile([128, 2 * TQ], F32, tag="s")
                    st["ps2"] = ps2
                    for half, ik in ((0, 2 * s), (1, 2 * s + 1)):
                        sl2 = slice(half * TQ, (half + 1) * TQ)
                        if half == 0:
                            # stationary K on partitions 0:64, moving Q 0:64
                            nc.tensor.matmul(
                                ps2[:, sl2],
                                kq_sb[0:64, ik * 128:(ik + 1) * 128],
                                qk_sb[0:64, sl],
                                start=True, stop=True,
                                tile_position=(0, 0))
                        else:
                            # stationary K on partitions 64:128, moving Q 64:128
                            nc.tensor.matmul(
                                ps2[:, sl2],
                                qk_sb[64:128, ik * 128:(ik + 1) * 128],
                                kq_sb[64:128, sl],
                                start=True, stop=True,
                                tile_position=(64, 0))

                def exp_pair(s):
                    eT = et_pool.tile([128, 2 * TQ], DT16, tag="et", name="eT")
                    st["eT"] = eT
                    nc.scalar.activation(eT[:], st["ps2"][:], AF.Exp)
                    if 2 * s >= jq * 4:
                        nc.vector.tensor_mul(
                            eT[:], eT[:],
                            masks01_sb[:, (2 * s - jq * 4) // 2, :])

                def pv_pair(s):
                    ik_a, ik_b = 2 * s, 2 * s + 1
                    eT = st["eT"]
                    nc.tensor.matmul(st["po"], v_sb[:, ik_a, :], eT[:, 0:TQ],
                                     start=(ik_a == 0), stop=False)
                    nc.tensor.matmul(st["po"], v_sb[:, ik_b, :],
                                     eT[:, TQ:2 * TQ],
                                     start=False, stop=(ik_b == n_ik - 1))

                for s in range(n_ik // 2):
                    th.append(lambda s=s: (s_pair(s), exp_pair(s)))
                    th.append(lambda s=s: pv_pair(s))

                def yout():
                    y_sb = y_pool.tile([H + 1, TQ], DT16, tag="ysb")
                    nc.vector.tensor_copy(y_sb[:], st["po"])
                    nc.gpsimd.dma_start(y_d[:, sl], y_sb[:])
                th.append(yout)
                return th

            def weave(a, b, f=1.0):
                out, i, j = [], 0, 0
                na, nb = len(a), len(b)
                while i < na or j < nb:
                    if j >= nb or (i < na and i * nb <= j * na * f):
                        out.append(a[i]); i += 1
                    else:
                        out.append(b[j]); j += 1
                return out

            # ---- schedule ----------------------------------------------
            # NOTE: emission order IS program order per engine; attention
            # for chunk jq must be emitted strictly after the projections
            # (and V transposes) it consumes.
            dma_x(0)()
            dma_x(1)()
            warmup()
            for t in qk_thunks(0) + vpair_thunks(0):
                t()
            waves = [
                ([dma_x(2), dma_x(3)] + qk_thunks(1), attn_thunks(0)),
                (qk_thunks(2) + vpair_thunks(1), attn_thunks(1)),
                (qk_thunks(3), attn_thunks(2)),
                ([], attn_thunks(3)),
            ]
            for pa, aa in waves:
                for t in weave(pa, aa):
                    t()

    nc.compile()
    return nc


def prepare_in_maps(x, wq, bq, wk, bk, wv, bv):
    f16 = np.float16
    f32 = np.float32
    sc = f32(C) ** -0.5
    x = np.asarray(x)
    # wave-major prepack: xw[p, w, c, t] = x[b, w*512+t, c*128+p]
    xw = x.astype(f16).reshape(B, N_JQ, TQ, N_KC, 128).transpose(0, 4, 1, 3, 2)
    wqk = np.concatenate([np.asarray(wq) * sc, np.asarray(wk)], 1).astype(f16)
    wqk = np.ascontiguousarray(wqk.reshape(N_KC, 128, 128).transpose(1, 0, 2))
    wv_c = np.asarray(wv).astype(f16)
    wv_c = np.ascontiguousarray(wv_c.reshape(N_KC, 128, H).transpose(1, 0, 2))
    bqk = np.concatenate([np.asarray(bq) * sc, np.asarray(bk)]) \
        .astype(f32).reshape(128, 1)
    bv_c = np.concatenate([np.asarray(bv), np.asarray(bv)]) \
        .astype(f32).reshape(128, 1)
    tk_i = np.arange(128)[:, None]
    tq_i = np.arange(TQ)[None, :]
    single = [(tq_i >= tk_i + m * 128) for m in range(4)]
    masks01 = np.ascontiguousarray(np.stack(
        [np.concatenate([single[0], single[1]], 1),
         np.concatenate([single[2], single[3]], 1)], 1).astype(f16))
    ident = np.ascontiguousarray(
        np.concatenate([np.eye(H), np.eye(H)], 0).astype(f16))
    shared = {"wqk": wqk, "wv": wv_c, "bqk": bqk, "bv": bv_c,
              "masks01": masks01, "ident": ident}
    return [{"xw": np.ascontiguousarray(xw[b]), **shared} for b in range(B)]


def postprocess(ys):
    out = np.empty((B, T, H), np.float32)
    for b, y in enumerate(ys):
        yf = y.astype(np.float32)
        out[b] = (yf[:H] / yf[H:H + 1]).T
    return out


def kernel(**inputs):
    global _CACHED_NC
    if _CACHED_NC is None:
        _CACHED_NC = build_program(reps=1)
    nc = _CACHED_NC
    in_maps = prepare_in_maps(
        inputs["x"], inputs["wq"], inputs["bq"], inputs["wk"],
        inputs["bk"], inputs["wv"], inputs["bv"])
    res = run_bass_kernel_spmd(nc, in_maps, core_ids=list(range(N_CORES)))
    return postprocess([r["y"] for r in res.results])


if __name__ == "__main__":
    rng = np.random.default_rng(0)
    demo = {
        "x": rng.standard_normal((B, T, C), dtype=np.float32),
        "wq": rng.standard_normal((C, H), dtype=np.float32) * 0.02,
        "bq": rng.standard_normal((H,), dtype=np.float32) * 0.02,
        "wk": rng.standard_normal((C, H), dtype=np.float32) * 0.02,
        "bk": rng.standard_normal((H,), dtype=np.float32) * 0.02,
        "wv": rng.standard_normal((C, H), dtype=np.float32) * 0.02,
        "bv": rng.standard_normal((H,), dtype=np.float32) * 0.02,
    }
    out = kernel(**demo)
    print("kernel output:", out.shape, out.dtype)


# revision 7
# speedup vs baseline: 1.1570x; 1.1437x over previous
"""Single-head causal attention on 8 Trainium2 NeuronCores.

Problem: x[8, 2048, 1024] f32, per-head projections (H=64), causal
softmax attention scaled by C**-0.5.

Strategy: data-parallel over batch (1 batch element per core). Per core
everything is kept in transposed layout so no fp32 on-chip transposes of
large tensors are needed:

  - host pre-casts x to fp16 and prepacks to wave-major xw
    [128, 4 waves, 8 c-chunks, 512 tok] so each wave loads as one DMA
    with 8KB contiguous partition lines (vs 1KB lines = 2-5x slower)
  - ~24 dummy matmuls sized to end when the first x data lands keep the
    PE HAM clock-gate warm (K=8/8, 2.4 GHz) for the real projections
  - Q^T/K^T [64, T] computed with [wq|wk] as stationary into one PSUM
    tile via PE column tiling; epilogue writes qk=[Q;K] in one 128-lane
    DVE op (bias fused), plus two small copies to build kq=[K;Q] for
    the row-tiled S matmuls
  - V^T for two token chunks is computed in one column-tiled pass
    (different moving streams -> both column groups run concurrently)
  - kernel pipelined over T-chunks of 512: projections for chunk tj
    weave with attention for query chunk tj-1
  - S^T tiles [128 Tk, 512 Tq] = (K^T chunk).T @ Q^T, two at a time via
    PE row tiling into one two-bank PSUM tile, single ScalarE exp per
    pair; causal mask via 0/1 mask multiply on VectorE
  - P@V: V tiles [128 Tk, 65] carry an appended ones column, so the
    softmax denominator falls out of the same PSUM accumulation
  - y returned as f16 [65, T]; normalization + transpose on host

Outputs are returned as float32 [8, 2048, 64].
"""

import numpy as np

import concourse.bass as bass
import concourse.mybir as mybir
import concourse.tile as tile
from concourse import bacc
from concourse.bass_utils import run_bass_kernel_spmd

B, T, C, H = 8, 2048, 1024, 64
N_CORES = 8
TQ = 512          # Tq chunk (one fp32 PSUM bank)
N_JQ = T // TQ    # 4
N_TK = T // 128   # 16
N_KC = C // 128   # 8  contraction chunks for projections

DT16 = mybir.dt.float16
F32 = mybir.dt.float32
AF = mybir.ActivationFunctionType

W_WARM = 24       # HAM warmup dummy matmuls (512 cols each)

_CACHED_NC = None


def build_program(reps=1):
    nc = bacc.Bacc("TRN2", target_bir_lowering=False, debug=False,
                   num_devices=N_CORES)

    xw_d = nc.dram_tensor("xw", [128, N_JQ, N_KC, TQ], DT16,
                          kind="ExternalInput").ap()
    wqk_d = nc.dram_tensor("wqk", [128, N_KC, 128], DT16,
                           kind="ExternalInput").ap()
    wv_d = nc.dram_tensor("wv", [128, N_KC, H], DT16,
                          kind="ExternalInput").ap()
    bqk_d = nc.dram_tensor("bqk", [128, 1], F32, kind="ExternalInput").ap()
    bv_d = nc.dram_tensor("bv", [128, 1], F32, kind="ExternalInput").ap()
    # 0/1 causal masks for the two diagonal pairs, [128, 2, 2*TQ]
    masks01_d = nc.dram_tensor("masks01", [128, 2, 2 * TQ], DT16,
                               kind="ExternalInput").ap()
    ident_d = nc.dram_tensor("ident", [128, H], DT16, kind="ExternalInput").ap()
    y_d = nc.dram_tensor("y", [H + 1, T], DT16, kind="ExternalOutput").ap()

    with tile.TileContext(nc) as tc:
        with (
            tc.tile_pool(name="const", bufs=1) as const,
            tc.tile_pool(name="data", bufs=1) as data,
            tc.tile_pool(name="et", bufs=6) as et_pool,
            tc.tile_pool(name="ysb", bufs=2) as y_pool,
            tc.tile_pool(name="ps_proj", bufs=2, space="PSUM") as ps_proj,
            tc.tile_pool(name="ps_s", bufs=2, space="PSUM") as ps_s,
            tc.tile_pool(name="ps_o", bufs=2, space="PSUM") as ps_o,
        ):
            # ---- constants ----------------------------------------------
            wqk_sb = const.tile([128, N_KC, 128], DT16, tag="wqk")
            wv_sb = const.tile([128, N_KC, H], DT16, tag="wv")
            bqk_sb = const.tile([128, 1], F32, tag="bqk")
            bv_sb = const.tile([128, 1], F32, tag="bv")
            masks01_sb = const.tile([128, 2, 2 * TQ], DT16, tag="masks01")
            ident_sb = const.tile([128, H], DT16, tag="ident")
            warm_sb = const.tile([128, TQ], DT16, tag="warm")

            # const loads: gpsimd queue (otherwise idle); priority order.
            # masks01 (512KB) rides the vector queue so it overlaps the
            # gpsimd consts and is ready for the early exp of attn(0).
            nc.gpsimd.dma_start(wqk_sb[:], wqk_d)
            nc.scalar.dma_start(masks01_sb[:], masks01_d)
            nc.gpsimd.dma_start(wv_sb[:], wv_d)
            nc.gpsimd.dma_start(ident_sb[:], ident_d)
            nc.gpsimd.dma_start(bqk_sb[:], bqk_d)
            nc.gpsimd.dma_start(bv_sb[:], bv_d)

            # ---- per-iteration tiles ------------------------------------
            # wave-major x: [128, wave, c, tok]
            xT_sb = data.tile([128, N_JQ, N_KC, TQ], DT16, tag="xT")
            qk_sb = data.tile([128, T], DT16, tag="qk")   # rows 0:64 Q, 64:128 K
            kq_sb = data.tile([128, T], DT16, tag="kq")   # rows 0:64 K, 64:128 Q
            vT_sb = data.tile([128, 2, TQ], DT16, tag="vT")
            v_sb = data.tile([128, N_TK, H + 1], DT16, tag="v")
            nc.vector.memset(v_sb[:], 1.0)
            nc.vector.memset(warm_sb[:], 0.5)

            # ---- x DMAs -------------------------------------------------
            def dma_x(wave):
                def go():
                    if wave == 0:
                        nc.sync.dma_start(xT_sb[:, 0, 0:4], xw_d[:, 0, 0:4])
                        nc.scalar.dma_start(xT_sb[:, 0, 4:8], xw_d[:, 0, 4:8])
                    elif wave == 2:
                        nc.scalar.dma_start(xT_sb[:, wave], xw_d[:, wave])
                    else:
                        nc.sync.dma_start(xT_sb[:, wave], xw_d[:, wave])
                return go

            # ---- HAM warmup: keep PE busy until first x data lands ------
            def warmup():
                wps = ps_proj.tile([128, TQ], F32, tag="proj", name="warm_ps")
                for _w in range(W_WARM):
                    nc.tensor.matmul(wps[:], warm_sb[:, 0:128], warm_sb[:],
                                     start=True, stop=True,
                                     skip_group_check=True)

            # ---- projections -------------------------------------------
            def qk_thunks(tj):
                sl = slice(tj * TQ, (tj + 1) * TQ)
                st = {}
                th = []

                def qk_mm(c):
                    if c == 0:
                        st["ps"] = ps_proj.tile([128, TQ], F32, tag="proj",
                                                name="ps_qk")
                    ps = st["ps"]
                    nc.tensor.matmul(ps[0:64, :], wqk_sb[:, c, 0:64],
                                     xT_sb[:, tj, c, :],
                                     start=(c == 0), stop=(c == N_KC - 1),
                                     tile_position=(0, 0),
                                     skip_group_check=True)
                    nc.tensor.matmul(ps[64:128, :], wqk_sb[:, c, 64:128],
                                     xT_sb[:, tj, c, :],
                                     start=(c == 0), stop=(c == N_KC - 1),
                                     tile_position=(0, 64),
                                     skip_group_check=True)
                for c in range(N_KC):
                    th.append(lambda c=c: qk_mm(c))

                def qk_epi():
                    nc.vector.tensor_scalar_add(qk_sb[:, sl], st["ps"][:],
                                                bqk_sb[:])
                    nc.vector.tensor_copy(kq_sb[0:64, sl], qk_sb[64:128, sl])
                    nc.vector.tensor_copy(kq_sb[64:128, sl], qk_sb[0:64, sl])
                th.append(qk_epi)
                return th

            def vpair_thunks(p):
                # V for token chunks 2p (rows 0:64) and 2p+1 (rows 64:128)
                st = {}
                th = []

                def v_mm(c):
                    if c == 0:
                        st["psv"] = ps_proj.tile([128, TQ], F32, tag="proj",
                                                 name="ps_v")
                    nc.tensor.matmul(st["psv"][0:64, :], wv_sb[:, c, :],
                                     xT_sb[:, 2 * p, c, :],
                                     start=(c == 0), stop=(c == N_KC - 1),
                                     tile_position=(0, 0),
                                     skip_group_check=True)
                    nc.tensor.matmul(st["psv"][64:128, :], wv_sb[:, c, :],
                                     xT_sb[:, 2 * p + 1, c, :],
                                     start=(c == 0), stop=(c == N_KC - 1),
                                     tile_position=(0, 64),
                                     skip_group_check=True)
                for c in range(N_KC):
                    th.append(lambda c=c: v_mm(c))

                def v_epi():
                    nc.vector.tensor_scalar_add(vT_sb[:, p, :], st["psv"][:],
                                                bv_sb[:])
                th.append(v_epi)

                def v_tr(r):
                    i = p * 8 + r
                    half = r // 4       # 0: chunk 2p, 1: chunk 2p+1
                    src = vT_sb[64 * half:64 * half + 64, p,
                                (r % 4) * 128:(r % 4 + 1) * 128]
                    pt = ps_proj.tile([128, H], DT16, tag="proj")
                    nc.tensor.transpose(
                        pt, src, ident_sb[64 * half:64 * half + 64, :])
                    nc.vector.tensor_copy(v_sb[:, i, 0:H], pt)
                for r in range(8):
                    th.append(lambda r=r: v_tr(r))
                return th

            # ---- attention ---------------------------------------------
            def attn_thunks(jq):
                sl = slice(jq * TQ, (jq + 1) * TQ)
                n_ik = (jq + 1) * 4
                st = {}
                th = []

                def s_pair(s):
                    if s == 0:
                        st["po"] = ps_o.tile([H + 1, TQ], F32, tag="out",
                                             name="po")
                    ps2 = ps_s.tile([128, 2 * TQ], F32, tag="s")
                    st["ps2"] = ps2
                    for half, ik in ((0, 2 * s), (1, 2 * s + 1)):
                        sl2 = slice(half * TQ, (half + 1) * TQ)
                        if half == 0:
                            # stationary K on partitions 0:64, moving Q 0:64
                            nc.tensor.matmul(
                                ps2[:, sl2],
                                kq_sb[0:64, ik * 128:(ik + 1) * 128],
                                qk_sb[0:64, sl],
                                start=True, stop=True,
                                tile_position=(0, 0))
                        else:
                            # stationary K on partitions 64:128, moving Q 64:128
                            nc.tensor.matmul(
                                ps2[:, sl2],
                                qk_sb[64:128, ik * 128:(ik + 1) * 128],
                                kq_sb[64:128, sl],
                                start=True, stop=True,
                                tile_position=(64, 0))

                def exp_pair(s):
                    eT = et_pool.tile([128, 2 * TQ], DT16, tag="et", name="eT")
                    st[f"eT{s}"] = eT
                    nc.scalar.activation(eT[:], st["ps2"][:], AF.Exp)
                    if 2 * s >= jq * 4:
                        nc.vector.tensor_mul(
                            eT[:], eT[:],
                            masks01_sb[:, (2 * s - jq * 4) // 2, :])

                def pv_pair(s):
                    ik_a, ik_b = 2 * s, 2 * s + 1
                    eT = st[f"eT{s}"]
                    nc.tensor.matmul(st["po"], v_sb[:, ik_a, :], eT[:, 0:TQ],
                                     start=(ik_a == 0), stop=False)
                    nc.tensor.matmul(st["po"], v_sb[:, ik_b, :],
                                     eT[:, TQ:2 * TQ],
                                     start=False, stop=(ik_b == n_ik - 1))

                for s in range(n_ik // 2):
                    th.append(lambda s=s: (s_pair(s), exp_pair(s)))
                    th.append(lambda s=s: pv_pair(s))

                def yout():
                    y_sb = y_pool.tile([H + 1, TQ], DT16, tag="ysb")
                    nc.vector.tensor_copy(y_sb[:], st["po"])
                    nc.gpsimd.dma_start(y_d[:, sl], y_sb[:])
                th.append(yout)
                return th

            def weave(a, b, f=1.0):
                out, i, j = [], 0, 0
                na, nb = len(a), len(b)
                while i < na or j < nb:
                    if j >= nb or (i < na and i * nb <= j * na * f):
                        out.append(a[i]); i += 1
                    else:
                        out.append(b[j]); j += 1
                return out

            # ---- schedule ----------------------------------------------
            # NOTE: emission order IS program order per engine; attention
            # for chunk jq must be emitted strictly after the projections
            # (and V transposes) it consumes.
            dma_x(0)()
            dma_x(1)()
            warmup()
            a0 = attn_thunks(0)   # [s0, pv0, s1, pv1, yout]
            vp0 = vpair_thunks(0)  # [v_mm x8, vepi, v_tr x8]
            vp1 = vpair_thunks(1)
            # pre-wave: QK(0), then attn(0)'s S/exp (start the ScalarE
            # chain early), then V(0,1) with only the transposes attn(0)
            # needs (tiles 0..3)
            for t in qk_thunks(0) + [a0[0], a0[2]] + vp0[:9] + vp0[9:13]:
                t()
            waves = [
                ([dma_x(2), dma_x(3)] + vp0[13:] + qk_thunks(1),
                 [a0[1], a0[3], a0[4]]),
                (qk_thunks(2) + vp1[:13], attn_thunks(1)),
                (vp1[13:] + qk_thunks(3), attn_thunks(2)),
                ([], attn_thunks(3)),
            ]
            for pa, aa in waves:
                for t in weave(pa, aa):
                    t()

    nc.compile()
    return nc


def prepare_in_maps(x, wq, bq, wk, bk, wv, bv):
    f16 = np.float16
    f32 = np.float32
    sc = f32(C) ** -0.5
    x = np.asarray(x)
    # wave-major prepack: xw[p, w, c, t] = x[b, w*512+t, c*128+p]
    xw = x.astype(f16).reshape(B, N_JQ, TQ, N_KC, 128).transpose(0, 4, 1, 3, 2)
    wqk = np.concatenate([np.asarray(wq) * sc, np.asarray(wk)], 1).astype(f16)
    wqk = np.ascontiguousarray(wqk.reshape(N_KC, 128, 128).transpose(1, 0, 2))
    wv_c = np.asarray(wv).astype(f16)
    wv_c = np.ascontiguousarray(wv_c.reshape(N_KC, 128, H).transpose(1, 0, 2))
    bqk = np.concatenate([np.asarray(bq) * sc, np.asarray(bk)]) \
        .astype(f32).reshape(128, 1)
    bv_c = np.concatenate([np.asarray(bv), np.asarray(bv)]) \
        .astype(f32).reshape(128, 1)
    tk_i = np.arange(128)[:, None]
    tq_i = np.arange(TQ)[None, :]
    single = [(tq_i >= tk_i + m * 128) for m in range(4)]
    masks01 = np.ascontiguousarray(np.stack(
        [np.concatenate([single[0], single[1]], 1),
         np.concatenate([single[2], single[3]], 1)], 1).astype(f16))
    ident = np.ascontiguousarray(
        np.concatenate([np.eye(H), np.eye(H)], 0).astype(f16))
    shared = {"wqk": wqk, "wv": wv_c, "bqk": bqk, "bv": bv_c,
              "masks01": masks01, "ident": ident}
    return [{"xw": np.ascontiguousarray(xw[b]), **shared} for b in range(B)]


def postprocess(ys):
    out = np.empty((B, T, H), np.float32)
    for b, y in enumerate(ys):
        yf = y.astype(np.float32)
        out[b] = (yf[:H] / yf[H:H + 1]).T
    return out


def kernel(**inputs):
    global _CACHED_NC
    if _CACHED_NC is None:
        _CACHED_NC = build_program(reps=1)
    nc = _CACHED_NC
    in_maps = prepare_in_maps(
        inputs["x"], inputs["wq"], inputs["bq"], inputs["wk"],
        inputs["bk"], inputs["wv"], inputs["bv"])
    res = run_bass_kernel_spmd(nc, in_maps, core_ids=list(range(N_CORES)))
    return postprocess([r["y"] for r in res.results])


if __name__ == "__main__":
    rng = np.random.default_rng(0)
    demo = {
        "x": rng.standard_normal((B, T, C), dtype=np.float32),
        "wq": rng.standard_normal((C, H), dtype=np.float32) * 0.02,
        "bq": rng.standard_normal((H,), dtype=np.float32) * 0.02,
        "wk": rng.standard_normal((C, H), dtype=np.float32) * 0.02,
        "bk": rng.standard_normal((H,), dtype=np.float32) * 0.02,
        "wv": rng.standard_normal((C, H), dtype=np.float32) * 0.02,
        "bv": rng.standard_normal((H,), dtype=np.float32) * 0.02,
    }
    out = kernel(**demo)
    print("kernel output:", out.shape, out.dtype)
